# revision 9
# baseline (speedup 1.0000x reference)
"""Trainium2 Bass kernel for nn_CNNVectorForm (LeNet-style CNN, batch 8192).

Pipeline per core (data-parallel over batch, 1024 images/core):
  conv 5x5 VALID (1->20ch, 28->24)  -> 2x2 maxpool -> fc1(2880->500) + relu
  -> fc2(500->10) + softmax

Device formulation (v2, bf16 datapath):
  * All activations feature-major [features, batch]; batch rides the free
    dim (512 per tile).  Weights and activations are bf16 (1 cycle/row on
    the PE, half the HBM traffic of fp32); PSUM accumulation is fp32.
  * Conv as a Toeplitz matmul over merged 6-row gathers: per pooled row
    ip and column half jb one [96, nb] gather (6 input rows x 16 cols)
    feeds 4 matmuls (output row parity dr x column parity eo) with
    zero-padded stationaries T4[dr,eo] [96, 120].  Merging the rows cuts
    gather traffic 40% and halves the DMA instruction count vs per-row
    gathers.
  * 2x2 maxpool: scalar engine evacuates ps0/ps2 (PSUM->SBUF), vector
    does the two width maxes, gpsimd (idle otherwise) does the final
    height max, writing a1 in bf16.  Three engines run in parallel and
    each stays under the PE's 8-matmul-per-kb budget.
  * fc1 weights host-permuted to pooled-feature order; fc1 rides the conv
    loop skewed by SKEW blocks (4 dependency-free matmuls per quad).
  * conv bias folded into the fc1 bias on the host.
  * fc2 feature-major (4 accumulating K=125 matmuls), bias via vector
    tensor_scalar_add, PE-transpose 128-wide slices for the softmax,
    results staged in one [128, 40] tile -> single output DMA per tile.
"""

import numpy as np
import ml_dtypes

N, H, W = 8192, 28, 28
COUT, KS = 20, 5
NCORES = 8
NPC = N // NCORES  # images per core
PH = 12            # pooled rows
FC1_IN, FC1_OUT, FC2_OUT = 2880, 500, 10
MT, MTS = 4, 125   # fc1 M tiles
KB, KBS = 24, 120  # a1 feature blocks (one per (pooled row, column half))
SKEW = 4           # fc1 trails conv by SKEW blocks
GROWS = 96         # merged gather rows: 6 input rows x 16 cols

BF16 = ml_dtypes.bfloat16

_cache = {}


def _build(npc, nb):
    from contextlib import ExitStack

    import concourse.tile as tile
    from concourse import bacc, mybir

    f32 = mybir.dt.float32
    bf16 = mybir.dt.bfloat16
    nbt = npc // nb

    nc = bacc.Bacc(
        "TRN2",
        target_bir_lowering=False,
        debug=False,
        enable_asserts=False,
        num_devices=NCORES,
    )

    # host-im2col'd input: xg[jb, p, ip, b] = x[(2*ip + p//16)*28 + 12*jb + p%16, b]
    xg_d = nc.dram_tensor(
        "xg", [2, GROWS, PH, npc], bf16, kind="ExternalInput"
    ).ap()
    t4_d = nc.dram_tensor("t4", [GROWS, 4 * KBS], bf16, kind="ExternalInput").ap()
    w1_d = nc.dram_tensor(
        "w1", [KB // 4, KBS, 4 * FC1_OUT], bf16, kind="ExternalInput"
    ).ap()
    b1_d = nc.dram_tensor("b1", [MTS, MT], f32, kind="ExternalInput").ap()
    w2_d = nc.dram_tensor("w2", [MTS, MT * FC2_OUT], bf16, kind="ExternalInput").ap()
    b2_d = nc.dram_tensor("b2", [FC2_OUT, 1], f32, kind="ExternalInput").ap()
    o_d = nc.dram_tensor("out", [npc, FC2_OUT], f32, kind="ExternalOutput").ap()

    with tile.TileContext(nc) as tc, ExitStack() as ctx:
        const = ctx.enter_context(tc.tile_pool(name="const", bufs=1))
        w1pool = ctx.enter_context(tc.tile_pool(name="w1", bufs=6))
        gpool = ctx.enter_context(tc.tile_pool(name="gather", bufs=8))
        a1pool = ctx.enter_context(tc.tile_pool(name="a1", bufs=SKEW + 4))
        tmppool = ctx.enter_context(tc.tile_pool(name="ptmp", bufs=6))
        a2pool = ctx.enter_context(tc.tile_pool(name="a2", bufs=2 * MT))
        smpool = ctx.enter_context(tc.tile_pool(name="softmax", bufs=4))
        cpsum = ctx.enter_context(tc.tile_pool(name="cpsum", bufs=4, space="PSUM"))
        fpsum = ctx.enter_context(tc.tile_pool(name="fpsum", bufs=4, space="PSUM"))

        from concourse.masks import make_identity

        # conv stationaries first on sync so the first matmul can start ASAP
        t4 = const.tile([GROWS, 4 * KBS], bf16)
        nc.sync.dma_start(t4[:], t4_d[:])
        # First conv gathers issued across all three DMA-capable engine
        # queues in parallel -- a single queue takes ~600ns per DMA issue
        # and the first matmuls are gated on gather arrival.
        pre_g = []
        pre_eng = [nc.scalar, nc.gpsimd, nc.sync, nc.scalar]
        for kb in range(SKEW):
            ip, jb = kb // 2, kb % 2
            g = gpool.tile([GROWS, nb], bf16, tag="g")
            pre_eng[kb].dma_start(g[:], xg_d[jb, :, ip, 0:nb])
            pre_g.append(g)
        # fc1 weights: 6 grouped DMAs, host-packed so every group is one
        # fully-contiguous [120, 2000] transfer, streamed from gpsimd's
        # queue so they don't block the gather stream on sync.
        WG = 4
        w1g = []
        for gidx in range(KB // WG):
            wt = w1pool.tile([KBS, WG * FC1_OUT], bf16, tag="w1",
                             name=f"w1g{gidx}")
            nc.gpsimd.dma_start(wt[:], w1_d[gidx])
            w1g.append(wt)
        b1t = const.tile([MTS, MT], f32)
        nc.scalar.dma_start(b1t[:], b1_d[:])
        w2t = const.tile([MTS, MT * FC2_OUT], bf16)
        nc.scalar.dma_start(w2t[:], w2_d[:])
        b2t = const.tile([FC2_OUT, 1], f32)
        nc.scalar.dma_start(b2t[:], b2_d[:])
        ident = const.tile([FC2_OUT, FC2_OUT], f32)
        make_identity(nc, ident[:])

        def w1_slice(j, mt):
            return w1g[j // WG][
                :, (j % WG) * FC1_OUT + mt * MTS : (j % WG) * FC1_OUT + (mt + 1) * MTS
            ]

        # PE warmup: HAM un-throttles the PE clock (1.2 -> 2.4 GHz) only
        # after ~3.4us of sustained matmul activity.  Real data needs
        # ~12us of DMA before the first conv matmul; a memset operand
        # gives the PE junk matmuls to chew on meanwhile so every real
        # matmul runs at full clock.
        warm = const.tile([128, nb], bf16)
        nc.vector.memset(warm[:], 0.0)
        for wi in range(16):
            wps = cpsum.tile([min(128, nb), nb], f32, tag="cps",
                             name=f"warm{wi}")
            nc.tensor.matmul(wps[:], warm[:, 0 : min(128, nb)], warm[:],
                             start=True, stop=True)

        sub = min(128, nb)
        nsub = nb // sub

        def emit_tail(bt, stage_no, st):
            """Tail of tile bt, split in 3 stages so it interleaves with
            the next tile's conv-only (pool-paced) window."""
            b0 = bt * nb
            if stage_no == 0:
                st["a2"] = []
                for mt in range(MT):
                    a2 = a2pool.tile([MTS, nb], bf16, tag="a2")
                    nc.scalar.activation(
                        a2[:], st["fp"][mt][:],
                        mybir.ActivationFunctionType.Relu,
                        bias=b1t[:, mt : mt + 1],
                    )
                    st["a2"].append(a2)
            elif stage_no == 1:
                p2f = fpsum.tile([FC2_OUT, nb], f32, tag="fps",
                                 name=f"p2f_{bt}")
                for mt in range(MT):
                    nc.tensor.matmul(
                        p2f[:],
                        w2t[:, mt * FC2_OUT : (mt + 1) * FC2_OUT],
                        st["a2"][mt][:],
                        start=(mt == 0),
                        stop=(mt == MT - 1),
                    )
                s2 = smpool.tile([FC2_OUT, nb], f32, tag="s2")
                nc.vector.tensor_scalar_add(s2[:], p2f[:], b2t[:, 0:1])
                st["s2"] = s2
            else:
                stage = smpool.tile([sub, nsub * FC2_OUT], f32, tag="ot")
                for s in range(nsub):
                    tp = fpsum.tile([sub, FC2_OUT], f32, tag="fps",
                                    name=f"tp_{bt}_{s}")
                    nc.tensor.transpose(
                        tp[:], st["s2"][:, s * sub : (s + 1) * sub], ident[:]
                    )
                    e = smpool.tile([sub, FC2_OUT], f32, tag="e")
                    ssum = smpool.tile([sub, 1], f32, tag="ss")
                    nc.scalar.activation(
                        e[:], tp[:], mybir.ActivationFunctionType.Exp,
                        accum_out=ssum[:],
                    )
                    rinv = smpool.tile([sub, 1], f32, tag="ri")
                    nc.vector.reciprocal(rinv[:], ssum[:])
                    nc.vector.tensor_scalar_mul(
                        stage[:, s * FC2_OUT : (s + 1) * FC2_OUT], e[:], rinv[:]
                    )
                dst = o_d[b0 : b0 + nb, :].rearrange(
                    "(s p) c -> p s c", s=nsub, p=sub
                )
                src = stage[:].rearrange("p (s c) -> p s c", s=nsub,
                                         c=FC2_OUT)
                nc.sync.dma_start(dst, src)

        tails = {}
        for bt in range(nbt):
            b0 = bt * nb
            a1 = [None] * KB
            st = {}
            tails[bt] = st
            for kb in range(KB + SKEW):
                # inject the previous tile's fc2/softmax into this tile's
                # conv-only window (kb < SKEW, PE has 4-matmul slack)
                if bt > 0 and 1 <= kb <= 3:
                    emit_tail(bt - 1, kb - 1, tails[bt - 1])
                if kb >= SKEW:
                    j = kb - SKEW
                    if j == 0:
                        # fc1 accumulators; allocated here so the previous
                        # tile's tail (p2f/tp) can rotate out first
                        st["fp"] = [
                            fpsum.tile([MTS, nb], f32, tag="fps",
                                       name=f"fp{bt}_{mt}")
                            for mt in range(MT)
                        ]
                    for mt in range(MT):
                        nc.tensor.matmul(
                            st["fp"][mt][:],
                            w1_slice(j, mt),
                            a1[j][:],
                            start=(j == 0),
                            stop=(j == KB - 1),
                        )
                if kb >= KB:
                    continue
                ip, jb = kb // 2, kb % 2
                if bt == 0 and kb < SKEW:
                    g = pre_g[kb]
                else:
                    g = gpool.tile([GROWS, nb], bf16, tag="g")
                    nc.sync.dma_start(g[:], xg_d[jb, :, ip, b0 : b0 + nb])
                ps = [
                    cpsum.tile([KBS, nb], f32, tag="cps", name=f"cps{i}")
                    for i in range(4)
                ]
                for dr in range(2):
                    for eo in range(2):
                        nc.tensor.matmul(
                            ps[2 * dr + eo][:],
                            t4[:, (2 * dr + eo) * KBS : (2 * dr + eo + 1) * KBS],
                            g[:],
                            start=True,
                            stop=True,
                        )
                # 2x2 maxpool: scalar evacuates the even-parity PSUM banks,
                # vector does the width maxes (PSUM-limited 1x) writing
                # bf16, so the final height max runs in the DVE's 2x_1P
                # packed mode at half cost.
                s0 = tmppool.tile([KBS, nb], f32, tag="s")
                nc.scalar.copy(s0[:], ps[0][:])
                m0 = tmppool.tile([KBS, nb], bf16, tag="m")
                nc.vector.tensor_max(m0[:], s0[:], ps[1][:])
                s1 = tmppool.tile([KBS, nb], f32, tag="s")
                nc.scalar.copy(s1[:], ps[2][:])
                m1 = tmppool.tile([KBS, nb], bf16, tag="m")
                nc.vector.tensor_max(m1[:], s1[:], ps[3][:])
                ab = a1pool.tile([KBS, nb], bf16, tag="a1")
                nc.vector.tensor_max(ab[:], m0[:], m1[:])
                a1[kb] = ab

        for stage_no in range(3):
            emit_tail(nbt - 1, stage_no, tails[nbt - 1])

    nc.compile()
    return nc


def _prep_weights(conv_w, conv_b, fc1_w, fc1_b, fc2_w, fc2_b):
    conv_w = np.asarray(conv_w, np.float32).reshape(COUT, KS, KS)
    conv_b = np.asarray(conv_b, np.float32)
    fc1_w = np.asarray(fc1_w, np.float32)
    fc1_b = np.asarray(fc1_b, np.float32)
    fc2_w = np.asarray(fc2_w, np.float32)
    fc2_b = np.asarray(fc2_b, np.float32)

    # Toeplitz conv matrices [96, 4*120]: four stationaries (dr, eo) over a
    # merged 6-row x 16-col gather; col m = (2*dr+eo)*120 + c*6 + q maps to
    # conv output (row 2*ip+dr, col 12*jb + 2*q+eo, channel c).
    T4 = np.zeros((GROWS, 4 * KBS), np.float32)
    for dr in range(2):
        for eo in range(2):
            for c in range(COUT):
                for q in range(6):
                    m = (2 * dr + eo) * KBS + c * 6 + q
                    for di in range(KS):
                        for dj in range(KS):
                            T4[(di + dr) * 16 + 2 * q + eo + dj, m] = conv_w[c, di, dj]

    # fc1 weights permuted to our pooled-feature order:
    # block kb = ip*2 + jb, within-block m = c*6 + q
    # -> original flat feature c*144 + ip*12 + jb*6 + q
    kbv = np.arange(KB)
    ipv, jbv = kbv // 2, kbv % 2
    ml = np.arange(KBS)
    cv, qv = ml // 6, ml % 6
    fidx = cv[None, :] * 144 + ipv[:, None] * 12 + jbv[:, None] * 6 + qv[None, :]
    w1 = fc1_w.T[fidx.reshape(-1)].reshape(KB, KBS, FC1_OUT)
    # pack into 6 contiguous groups of 4 blocks: [6, 120, 4*500]
    w1 = np.ascontiguousarray(
        w1.reshape(KB // 4, 4, KBS, FC1_OUT).transpose(0, 2, 1, 3)
    ).reshape(KB // 4, KBS, 4 * FC1_OUT)

    # conv bias folded into fc1 bias (pool-max commutes with per-channel const)
    cb_vec = np.repeat(conv_b, 144)
    b1p = fc1_b + fc1_w @ cb_vec
    b1 = np.ascontiguousarray(b1p.reshape(MT, MTS).T)

    w2 = np.ascontiguousarray(
        fc2_w.T.reshape(MT, MTS, FC2_OUT).transpose(1, 0, 2)
    ).reshape(MTS, MT * FC2_OUT)
    b2 = np.ascontiguousarray(fc2_b.reshape(FC2_OUT, 1))
    return (T4.astype(BF16), w1.astype(BF16), b1,
            w2.astype(BF16), b2)


# im2col pixel indices: idx[jb, p, ip] = (2*ip + p//16)*28 + 12*jb + p%16
_IDX = np.zeros((2, GROWS, PH), np.int64)
for _jb in range(2):
    for _di in range(6):
        for _jjp in range(16):
            for _ip in range(PH):
                _IDX[_jb, _di * 16 + _jjp, _ip] = (2 * _ip + _di) * W + 12 * _jb + _jjp


def _prep_x(x_core):
    """x_core [784, npc] pixel-major -> xg [2, 96, 12, npc] bf16."""
    return np.ascontiguousarray(x_core[_IDX.reshape(-1)].reshape(
        2, GROWS, PH, x_core.shape[1]).astype(BF16))


def _feeds(inputs, npc):
    """Per-core feed dicts for the full batch (list of NCORES dicts)."""
    T4, w1, b1, w2, b2 = _prep_weights(
        inputs["conv_w"], inputs["conv_b"], inputs["fc1_w"],
        inputs["fc1_b"], inputs["fc2_w"], inputs["fc2_b"],
    )
    x = np.asarray(inputs["x"], np.float32).reshape(-1, H * W)
    n_total = x.shape[0]
    assert n_total == NCORES * npc
    xs = x.reshape(NCORES, npc, H * W).transpose(0, 2, 1)
    return [
        {"xg": _prep_x(xs[i]), "t4": T4, "w1": w1, "b1": b1, "w2": w2,
         "b2": b2}
        for i in range(NCORES)
    ]


def _run(inputs, npc=NPC, nb=512, trace=False):
    from concourse import bass_utils

    key = (npc, nb)
    if key not in _cache:
        _cache[key] = _build(npc, nb)
    nc = _cache[key]

    in_maps = _feeds(inputs, npc)
    res = bass_utils.run_bass_kernel_spmd(
        nc, in_maps, core_ids=list(range(NCORES)), trace=trace
    )
    out = np.concatenate([res.results[i]["out"] for i in range(NCORES)], axis=0)
    return out, res


def kernel(**inputs):
    out, _ = _run(inputs)
    return out


# revision 16
# speedup vs baseline: 1.0239x; 1.0239x over previous
"""Trainium2 Bass kernel for nn_CNNVectorForm (LeNet-style CNN, batch 8192).

Pipeline per core (data-parallel over batch, 1024 images/core):
  conv 5x5 VALID (1->20ch, 28->24)  -> 2x2 maxpool -> fc1(2880->500) + relu
  -> fc2(500->10) + softmax

Device formulation (v2, bf16 datapath):
  * All activations feature-major [features, batch]; batch rides the free
    dim (512 per tile).  Weights and activations are bf16 (1 cycle/row on
    the PE, half the HBM traffic of fp32); PSUM accumulation is fp32.
  * Conv as a Toeplitz matmul over merged 6-row gathers: per pooled row
    ip and column half jb one [96, nb] gather (6 input rows x 16 cols)
    feeds 4 matmuls (output row parity dr x column parity eo) with
    zero-padded stationaries T4[dr,eo] [96, 120].  Merging the rows cuts
    gather traffic 40% and halves the DMA instruction count vs per-row
    gathers.
  * 2x2 maxpool: scalar engine evacuates ps0/ps2 (PSUM->SBUF), vector
    does the two width maxes, gpsimd (idle otherwise) does the final
    height max, writing a1 in bf16.  Three engines run in parallel and
    each stays under the PE's 8-matmul-per-kb budget.
  * fc1 weights host-permuted to pooled-feature order; fc1 rides the conv
    loop skewed by SKEW blocks (4 dependency-free matmuls per quad).
  * conv bias folded into the fc1 bias on the host.
  * fc2 feature-major (4 accumulating K=125 matmuls), bias via vector
    tensor_scalar_add, PE-transpose 128-wide slices for the softmax,
    results staged in one [128, 40] tile -> single output DMA per tile.
"""

import numpy as np
import ml_dtypes

N, H, W = 8192, 28, 28
COUT, KS = 20, 5
NCORES = 8
NPC = N // NCORES  # images per core
PH = 12            # pooled rows
FC1_IN, FC1_OUT, FC2_OUT = 2880, 500, 10
MT, MTS = 4, 125   # fc1 M tiles
KB, KBS = 24, 120  # a1 feature blocks (one per (pooled row, column half))
SKEW = 4           # fc1 trails conv by SKEW blocks
GROWS = 96         # merged gather rows: 6 input rows x 16 cols

BF16 = ml_dtypes.bfloat16

_cache = {}


def _build(npc, nb):
    from contextlib import ExitStack

    import concourse.tile as tile
    from concourse import bacc, mybir

    f32 = mybir.dt.float32
    bf16 = mybir.dt.bfloat16
    nbt = npc // nb

    nc = bacc.Bacc(
        "TRN2",
        target_bir_lowering=False,
        debug=False,
        enable_asserts=False,
        num_devices=NCORES,
    )

    # host-im2col'd input: xg[jb, p, ip, b] = x[(2*ip + p//16)*28 + 12*jb + p%16, b]
    xg_d = nc.dram_tensor(
        "xg", [2, GROWS, PH, npc], bf16, kind="ExternalInput"
    ).ap()
    t4_d = nc.dram_tensor("t4", [GROWS, 4 * KBS], bf16, kind="ExternalInput").ap()
    w1_d = nc.dram_tensor(
        "w1", [KB // 4, KBS, 4 * FC1_OUT], bf16, kind="ExternalInput"
    ).ap()
    b1_d = nc.dram_tensor("b1", [MTS, MT], f32, kind="ExternalInput").ap()
    w2_d = nc.dram_tensor("w2", [MTS, MT * FC2_OUT], bf16, kind="ExternalInput").ap()
    b2_d = nc.dram_tensor("b2", [FC2_OUT, 1], f32, kind="ExternalInput").ap()
    # output stays in the device-native [128, nbt*nsub*10] staging layout;
    # the host untangles it (fewer, bigger DMA descriptors)
    sub = min(128, nb)
    nsub = nb // sub
    o_d = nc.dram_tensor(
        "out", [sub, nbt * nsub * FC2_OUT], f32, kind="ExternalOutput"
    ).ap()

    with tile.TileContext(nc) as tc, ExitStack() as ctx:
        const = ctx.enter_context(tc.tile_pool(name="const", bufs=1))
        w1pool = ctx.enter_context(tc.tile_pool(name="w1", bufs=6))
        gpool = ctx.enter_context(tc.tile_pool(name="gather", bufs=1))
        a1pool = ctx.enter_context(tc.tile_pool(name="a1", bufs=SKEW + 4))
        tmppool = ctx.enter_context(tc.tile_pool(name="ptmp", bufs=6))
        a2pool = ctx.enter_context(tc.tile_pool(name="a2", bufs=2 * MT))
        smpool = ctx.enter_context(tc.tile_pool(name="softmax", bufs=4))
        cpsum = ctx.enter_context(tc.tile_pool(name="cpsum", bufs=4, space="PSUM"))
        fpsum = ctx.enter_context(tc.tile_pool(name="fpsum", bufs=4, space="PSUM"))

        from concourse.masks import make_identity

        # conv stationaries first on sync so the first matmul can start ASAP
        t4 = const.tile([GROWS, 4 * KBS], bf16)
        nc.sync.dma_start(t4[:], t4_d[:])
        # Whole-input prefetch: 24 double-width gathers (both batch tiles)
        # issued upfront.  2KB descriptor lines win DMA arbitration against
        # the weight stream, and tile 1's conv never touches HBM at all.
        gx = []
        for kb in range(KB):
            ip, jb = kb // 2, kb % 2
            g = gpool.tile([GROWS, npc], bf16, tag=f"g{kb}")
            nc.sync.dma_start(g[:], xg_d[jb, :, ip, :])
            gx.append(g)
        # fc1 weights: 6 grouped DMAs, host-packed so every group is one
        # fully-contiguous [120, 2000] transfer, streamed from gpsimd's
        # queue so they don't block the gather stream on sync.
        WG = 4
        w1g = []
        for gidx in range(KB // WG):
            wt = w1pool.tile([KBS, WG * FC1_OUT], bf16, tag="w1",
                             name=f"w1g{gidx}")
            nc.gpsimd.dma_start(wt[:], w1_d[gidx])
            w1g.append(wt)
        b1t = const.tile([MTS, MT], f32)
        nc.scalar.dma_start(b1t[:], b1_d[:])
        w2t = const.tile([MTS, MT * FC2_OUT], bf16)
        nc.scalar.dma_start(w2t[:], w2_d[:])
        b2t = const.tile([FC2_OUT, 1], f32)
        nc.scalar.dma_start(b2t[:], b2_d[:])
        ident = const.tile([FC2_OUT, FC2_OUT], f32)
        make_identity(nc, ident[:])

        def w1_slice(j, mt):
            return w1g[j // WG][
                :, (j % WG) * FC1_OUT + mt * MTS : (j % WG) * FC1_OUT + (mt + 1) * MTS
            ]

        # PE warmup: HAM un-throttles the PE clock (1.2 -> 2.4 GHz) only
        # after ~3.4us of sustained matmul activity.  Real data needs
        # ~12us of DMA before the first conv matmul; a memset operand
        # gives the PE junk matmuls to chew on meanwhile so every real
        # matmul runs at full clock.
        warm = const.tile([128, nb], bf16)
        nc.vector.memset(warm[:], 0.0)
        for wi in range(16):
            wps = cpsum.tile([min(128, nb), nb], f32, tag="cps",
                             name=f"warm{wi}")
            nc.tensor.matmul(wps[:], warm[:, 0 : min(128, nb)], warm[:],
                             start=True, stop=True)

        stage = const.tile([sub, nbt * nsub * FC2_OUT], f32)

        def emit_tail(bt, stage_no, st):
            """Tail of tile bt, split in 3 stages so it interleaves with
            the next tile's conv-only (pool-paced) window."""
            b0 = bt * nb
            if stage_no == 0:
                st["a2"] = []
                for mt in range(MT):
                    a2 = a2pool.tile([MTS, nb], bf16, tag="a2")
                    nc.scalar.activation(
                        a2[:], st["fp"][mt][:],
                        mybir.ActivationFunctionType.Relu,
                        bias=b1t[:, mt : mt + 1],
                    )
                    st["a2"].append(a2)
            elif stage_no == 1:
                p2f = fpsum.tile([FC2_OUT, nb], f32, tag="fps",
                                 name=f"p2f_{bt}")
                for mt in range(MT):
                    nc.tensor.matmul(
                        p2f[:],
                        w2t[:, mt * FC2_OUT : (mt + 1) * FC2_OUT],
                        st["a2"][mt][:],
                        start=(mt == 0),
                        stop=(mt == MT - 1),
                    )
                s2 = smpool.tile([FC2_OUT, nb], f32, tag="s2")
                nc.vector.tensor_scalar_add(s2[:], p2f[:], b2t[:, 0:1])
                st["s2"] = s2
            else:
                for s in range(nsub):
                    tp = fpsum.tile([sub, FC2_OUT], f32, tag="fps",
                                    name=f"tp_{bt}_{s}")
                    nc.tensor.transpose(
                        tp[:], st["s2"][:, s * sub : (s + 1) * sub], ident[:]
                    )
                    e = smpool.tile([sub, FC2_OUT], f32, tag="e")
                    nc.scalar.activation(
                        e[:], tp[:], mybir.ActivationFunctionType.Exp,
                    )
                    ssum = smpool.tile([sub, 1], f32, tag="ss")
                    nc.vector.tensor_reduce(
                        ssum[:], e[:], axis=mybir.AxisListType.X,
                        op=mybir.AluOpType.add,
                    )
                    rinv = smpool.tile([sub, 1], f32, tag="ri")
                    nc.vector.reciprocal(rinv[:], ssum[:])
                    c0 = (bt * nsub + s) * FC2_OUT
                    nc.vector.tensor_scalar_mul(
                        stage[:, c0 : c0 + FC2_OUT], e[:], rinv[:]
                    )
                if bt == nbt - 1:
                    nc.sync.dma_start(o_d[:], stage[:])

        tails = {}
        for bt in range(nbt):
            b0 = bt * nb
            a1 = [None] * KB
            st = {}
            tails[bt] = st
            for kb in range(KB + SKEW):
                # inject the previous tile's fc2/softmax into this tile's
                # conv-only window (kb < SKEW, PE has 4-matmul slack)
                if bt > 0 and 1 <= kb <= 3:
                    emit_tail(bt - 1, kb - 1, tails[bt - 1])
                if kb >= SKEW:
                    j = kb - SKEW
                    if j == 0:
                        # fc1 accumulators; allocated here so the previous
                        # tile's tail (p2f/tp) can rotate out first
                        st["fp"] = [
                            fpsum.tile([MTS, nb], f32, tag="fps",
                                       name=f"fp{bt}_{mt}")
                            for mt in range(MT)
                        ]
                    for mt in range(MT):
                        nc.tensor.matmul(
                            st["fp"][mt][:],
                            w1_slice(j, mt),
                            a1[j][:],
                            start=(j == 0),
                            stop=(j == KB - 1),
                        )
                if kb >= KB:
                    continue
                ps = [
                    cpsum.tile([KBS, nb], f32, tag="cps", name=f"cps{i}")
                    for i in range(4)
                ]
                for dr in range(2):
                    for eo in range(2):
                        nc.tensor.matmul(
                            ps[2 * dr + eo][:],
                            t4[:, (2 * dr + eo) * KBS : (2 * dr + eo + 1) * KBS],
                            gx[kb][:, b0 : b0 + nb],
                            start=True,
                            stop=True,
                        )
                # 2x2 maxpool: scalar evacuates the even-parity PSUM banks,
                # vector does the width maxes (PSUM-limited 1x) writing
                # bf16, so the final height max runs in the DVE's 2x_1P
                # packed mode at half cost.
                s0 = tmppool.tile([KBS, nb], f32, tag="s")
                nc.scalar.copy(s0[:], ps[0][:])
                m0 = tmppool.tile([KBS, nb], bf16, tag="m")
                nc.vector.tensor_max(m0[:], s0[:], ps[1][:])
                s1 = tmppool.tile([KBS, nb], f32, tag="s")
                nc.scalar.copy(s1[:], ps[2][:])
                m1 = tmppool.tile([KBS, nb], bf16, tag="m")
                nc.vector.tensor_max(m1[:], s1[:], ps[3][:])
                ab = a1pool.tile([KBS, nb], bf16, tag="a1")
                nc.vector.tensor_max(ab[:], m0[:], m1[:])
                a1[kb] = ab

        for stage_no in range(3):
            emit_tail(nbt - 1, stage_no, tails[nbt - 1])

    nc.compile()
    return nc


def _prep_weights(conv_w, conv_b, fc1_w, fc1_b, fc2_w, fc2_b):
    conv_w = np.asarray(conv_w, np.float32).reshape(COUT, KS, KS)
    conv_b = np.asarray(conv_b, np.float32)
    fc1_w = np.asarray(fc1_w, np.float32)
    fc1_b = np.asarray(fc1_b, np.float32)
    fc2_w = np.asarray(fc2_w, np.float32)
    fc2_b = np.asarray(fc2_b, np.float32)

    # Toeplitz conv matrices [96, 4*120]: four stationaries (dr, eo) over a
    # merged 6-row x 16-col gather; col m = (2*dr+eo)*120 + c*6 + q maps to
    # conv output (row 2*ip+dr, col 12*jb + 2*q+eo, channel c).
    T4 = np.zeros((GROWS, 4 * KBS), np.float32)
    for dr in range(2):
        for eo in range(2):
            for c in range(COUT):
                for q in range(6):
                    m = (2 * dr + eo) * KBS + c * 6 + q
                    for di in range(KS):
                        for dj in range(KS):
                            T4[(di + dr) * 16 + 2 * q + eo + dj, m] = conv_w[c, di, dj]

    # fc1 weights permuted to our pooled-feature order:
    # block kb = ip*2 + jb, within-block m = c*6 + q
    # -> original flat feature c*144 + ip*12 + jb*6 + q
    kbv = np.arange(KB)
    ipv, jbv = kbv // 2, kbv % 2
    ml = np.arange(KBS)
    cv, qv = ml // 6, ml % 6
    fidx = cv[None, :] * 144 + ipv[:, None] * 12 + jbv[:, None] * 6 + qv[None, :]
    w1 = fc1_w.T[fidx.reshape(-1)].reshape(KB, KBS, FC1_OUT)
    # pack into 6 contiguous groups of 4 blocks: [6, 120, 4*500]
    w1 = np.ascontiguousarray(
        w1.reshape(KB // 4, 4, KBS, FC1_OUT).transpose(0, 2, 1, 3)
    ).reshape(KB // 4, KBS, 4 * FC1_OUT)

    # conv bias folded into fc1 bias (pool-max commutes with per-channel const)
    cb_vec = np.repeat(conv_b, 144)
    b1p = fc1_b + fc1_w @ cb_vec
    b1 = np.ascontiguousarray(b1p.reshape(MT, MTS).T)

    w2 = np.ascontiguousarray(
        fc2_w.T.reshape(MT, MTS, FC2_OUT).transpose(1, 0, 2)
    ).reshape(MTS, MT * FC2_OUT)
    b2 = np.ascontiguousarray(fc2_b.reshape(FC2_OUT, 1))
    return (T4.astype(BF16), w1.astype(BF16), b1,
            w2.astype(BF16), b2)


# im2col pixel indices: idx[jb, p, ip] = (2*ip + p//16)*28 + 12*jb + p%16
_IDX = np.zeros((2, GROWS, PH), np.int64)
for _jb in range(2):
    for _di in range(6):
        for _jjp in range(16):
            for _ip in range(PH):
                _IDX[_jb, _di * 16 + _jjp, _ip] = (2 * _ip + _di) * W + 12 * _jb + _jjp


def _prep_x(x_core):
    """x_core [784, npc] pixel-major -> xg [2, 96, 12, npc] bf16."""
    return np.ascontiguousarray(x_core[_IDX.reshape(-1)].reshape(
        2, GROWS, PH, x_core.shape[1]).astype(BF16))


def _feeds(inputs, npc):
    """Per-core feed dicts for the full batch (list of NCORES dicts)."""
    T4, w1, b1, w2, b2 = _prep_weights(
        inputs["conv_w"], inputs["conv_b"], inputs["fc1_w"],
        inputs["fc1_b"], inputs["fc2_w"], inputs["fc2_b"],
    )
    x = np.asarray(inputs["x"], np.float32).reshape(-1, H * W)
    n_total = x.shape[0]
    assert n_total == NCORES * npc
    xs = x.reshape(NCORES, npc, H * W).transpose(0, 2, 1)
    return [
        {"xg": _prep_x(xs[i]), "t4": T4, "w1": w1, "b1": b1, "w2": w2,
         "b2": b2}
        for i in range(NCORES)
    ]


def _untangle_out(o, npc, nb):
    """Device staging layout [sub, nbt*nsub*10] -> [npc, 10]."""
    sub = min(128, nb)
    nsub = nb // sub
    nbt = npc // nb
    return np.ascontiguousarray(
        np.asarray(o).reshape(sub, nbt, nsub, FC2_OUT)
        .transpose(1, 2, 0, 3).reshape(npc, FC2_OUT)
    )


def _run(inputs, npc=NPC, nb=512, trace=False):
    from concourse import bass_utils

    key = (npc, nb)
    if key not in _cache:
        _cache[key] = _build(npc, nb)
    nc = _cache[key]

    in_maps = _feeds(inputs, npc)
    res = bass_utils.run_bass_kernel_spmd(
        nc, in_maps, core_ids=list(range(NCORES)), trace=trace
    )
    out = np.concatenate(
        [_untangle_out(res.results[i]["out"], npc, nb) for i in range(NCORES)],
        axis=0,
    )
    return out, res


def kernel(**inputs):
    out, _ = _run(inputs)
    return out


# revision 23
# speedup vs baseline: 1.0455x; 1.0211x over previous
"""Trainium2 Bass kernel for nn_CNNVectorForm (LeNet-style CNN, batch 8192).

Pipeline per core (data-parallel over batch, 1024 images/core):
  conv 5x5 VALID (1->20ch, 28->24)  -> 2x2 maxpool -> fc1(2880->500) + relu
  -> fc2(500->10) + softmax

Device formulation (v2, bf16 datapath):
  * All activations feature-major [features, batch]; batch rides the free
    dim (512 per tile).  Weights and activations are bf16 (1 cycle/row on
    the PE, half the HBM traffic of fp32); PSUM accumulation is fp32.
  * Conv as a Toeplitz matmul over merged 6-row gathers: per pooled row
    ip and column half jb one [96, nb] gather (6 input rows x 16 cols)
    feeds 4 matmuls (output row parity dr x column parity eo) with
    zero-padded stationaries T4[dr,eo] [96, 120].  Merging the rows cuts
    gather traffic 40% and halves the DMA instruction count vs per-row
    gathers.
  * 2x2 maxpool: scalar engine evacuates ps0/ps2 (PSUM->SBUF), vector
    does the two width maxes, gpsimd (idle otherwise) does the final
    height max, writing a1 in bf16.  Three engines run in parallel and
    each stays under the PE's 8-matmul-per-kb budget.
  * fc1 weights host-permuted to pooled-feature order; fc1 rides the conv
    loop skewed by SKEW blocks (4 dependency-free matmuls per quad).
  * conv bias folded into the fc1 bias on the host.
  * fc2 feature-major (4 accumulating K=125 matmuls), bias via vector
    tensor_scalar_add, PE-transpose 128-wide slices for the softmax,
    results staged in one [128, 40] tile -> single output DMA per tile.
"""

import numpy as np
import ml_dtypes

N, H, W = 8192, 28, 28
COUT, KS = 20, 5
NCORES = 8
NPC = N // NCORES  # images per core
PH = 12            # pooled rows
FC1_IN, FC1_OUT, FC2_OUT = 2880, 500, 10
MT, MTS = 4, 125   # fc1 M tiles
KB, KBS = 24, 120  # a1 feature blocks (one per (pooled row, column half))
SKEW = 6           # fc1 trails conv by SKEW blocks
GROWS = 96         # merged gather rows: 6 input rows x 16 cols

BF16 = ml_dtypes.bfloat16

_cache = {}


def _build(npc, nb):
    from contextlib import ExitStack

    import concourse.tile as tile
    from concourse import bacc, mybir

    f32 = mybir.dt.float32
    bf16 = mybir.dt.bfloat16
    nbt = npc // nb

    nc = bacc.Bacc(
        "TRN2",
        target_bir_lowering=False,
        debug=False,
        enable_asserts=False,
        num_devices=NCORES,
    )

    # host-im2col'd input: xg[jb, p, ip, b] = x[(2*ip + p//16)*28 + 12*jb + p%16, b]
    xg_d = nc.dram_tensor(
        "xg", [2, GROWS, PH, npc], bf16, kind="ExternalInput"
    ).ap()
    t4_d = nc.dram_tensor("t4", [GROWS, 4 * KBS], bf16, kind="ExternalInput").ap()
    w1_d = nc.dram_tensor(
        "w1", [KB // 4, KBS, 4 * FC1_OUT], bf16, kind="ExternalInput"
    ).ap()
    b1_d = nc.dram_tensor("b1", [MTS, MT], f32, kind="ExternalInput").ap()
    w2_d = nc.dram_tensor("w2", [MTS, MT * FC2_OUT], bf16, kind="ExternalInput").ap()
    b2_d = nc.dram_tensor("b2", [FC2_OUT, 1], f32, kind="ExternalInput").ap()
    # output stays in the device-native [128, nbt*nsub*10] staging layout;
    # the host untangles it (fewer, bigger DMA descriptors)
    sub = min(128, nb)
    nsub = nb // sub
    o_d = nc.dram_tensor(
        "out", [sub, nbt * nsub * FC2_OUT], f32, kind="ExternalOutput"
    ).ap()

    with tile.TileContext(nc) as tc, ExitStack() as ctx:
        const = ctx.enter_context(tc.tile_pool(name="const", bufs=1))
        w1pool = ctx.enter_context(tc.tile_pool(name="w1", bufs=6))
        gpool = ctx.enter_context(tc.tile_pool(name="gather", bufs=1))
        a1pool = ctx.enter_context(tc.tile_pool(name="a1", bufs=SKEW + 4))
        tmppool = ctx.enter_context(tc.tile_pool(name="ptmp", bufs=6))
        a2pool = ctx.enter_context(tc.tile_pool(name="a2", bufs=2 * MT))
        smpool = ctx.enter_context(tc.tile_pool(name="softmax", bufs=4))
        cpsum = ctx.enter_context(tc.tile_pool(name="cpsum", bufs=4, space="PSUM"))
        fpsum = ctx.enter_context(tc.tile_pool(name="fpsum", bufs=4, space="PSUM"))

        from concourse.masks import make_identity

        # The DMA system delivers only ~30-40 GB/s for the first ~10us, so
        # the issue ORDER is a priority list.  First the bytes that gate
        # the first conv matmul (half of t4, tile-0's first gather), then
        # tile-0's gather stream, then tile-1's.  w1 rides gpsimd's queue,
        # group 0 upfront and the rest rationed inside the loop right
        # before each is needed, so it can't crowd out the gathers.
        t4 = const.tile([GROWS, 4 * KBS], bf16)
        nc.sync.dma_start(t4[:, 0 : 2 * KBS], t4_d[:, 0 : 2 * KBS])
        nc.sync.dma_start(t4[:, 2 * KBS :], t4_d[:, 2 * KBS :])
        gx = [[None] * KB for _ in range(nbt)]
        for bt in range(nbt):
            for kb in range(KB):
                ip, jb = kb // 2, kb % 2
                g = gpool.tile([GROWS, nb], bf16, tag=f"g{bt}_{kb}")
                nc.sync.dma_start(
                    g[:], xg_d[jb, :, ip, bt * nb : (bt + 1) * nb]
                )
                gx[bt][kb] = g
        WG = 4
        w1g = []
        for gidx in range(KB // WG):
            wt = w1pool.tile([KBS, WG * FC1_OUT], bf16, tag="w1",
                             name=f"w1g{gidx}")
            if gidx == 0:
                nc.gpsimd.dma_start(wt[:], w1_d[gidx])
            w1g.append(wt)
        b1t = const.tile([MTS, MT], f32)
        nc.scalar.dma_start(b1t[:], b1_d[:])
        w2t = const.tile([MTS, MT * FC2_OUT], bf16)
        nc.scalar.dma_start(w2t[:], w2_d[:])
        b2t = const.tile([FC2_OUT, 1], f32)
        nc.scalar.dma_start(b2t[:], b2_d[:])
        ident = const.tile([FC2_OUT, FC2_OUT], f32)
        make_identity(nc, ident[:])

        def w1_slice(j, mt):
            return w1g[j // WG][
                :, (j % WG) * FC1_OUT + mt * MTS : (j % WG) * FC1_OUT + (mt + 1) * MTS
            ]

        # PE warmup: HAM un-throttles the PE clock (1.2 -> 2.4 GHz) only
        # after ~3.4us of sustained matmul activity.  Real data needs
        # ~12us of DMA before the first conv matmul; a memset operand
        # gives the PE junk matmuls to chew on meanwhile so every real
        # matmul runs at full clock.
        warm = const.tile([128, nb], bf16)
        nc.vector.memset(warm[:], 0.0)
        for wi in range(16):
            wps = cpsum.tile([min(128, nb), nb], f32, tag="cps",
                             name=f"warm{wi}")
            nc.tensor.matmul(wps[:], warm[:, 0 : min(128, nb)], warm[:],
                             start=True, stop=True)

        stage = const.tile([sub, nbt * nsub * FC2_OUT], f32)

        def emit_tail(bt, stage_no, st):
            """Tail of tile bt, split in 3 stages so it interleaves with
            the next tile's conv-only (pool-paced) window."""
            b0 = bt * nb
            if stage_no == 0:
                st["a2"] = []
                for mt in range(MT):
                    a2 = a2pool.tile([MTS, nb], bf16, tag="a2")
                    if bt == nbt - 1 and mt % 2 == 1:
                        # final tile: the relu chain is exposed tail
                        # latency; split it across scalar and vector
                        # (fused (fp + b1) max 0 in one DVE op)
                        nc.vector.tensor_scalar(
                            a2[:], st["fp"][mt][:],
                            b1t[:, mt : mt + 1], 0.0,
                            mybir.AluOpType.add, mybir.AluOpType.max,
                        )
                    else:
                        nc.scalar.activation(
                            a2[:], st["fp"][mt][:],
                            mybir.ActivationFunctionType.Relu,
                            bias=b1t[:, mt : mt + 1],
                        )
                    st["a2"].append(a2)
            elif stage_no == 1:
                p2f = fpsum.tile([FC2_OUT, nb], f32, tag="fps",
                                 name=f"p2f_{bt}")
                for mt in range(MT):
                    nc.tensor.matmul(
                        p2f[:],
                        w2t[:, mt * FC2_OUT : (mt + 1) * FC2_OUT],
                        st["a2"][mt][:],
                        start=(mt == 0),
                        stop=(mt == MT - 1),
                    )
                s2 = smpool.tile([FC2_OUT, nb], f32, tag="s2")
                nc.vector.tensor_scalar_add(s2[:], p2f[:], b2t[:, 0:1])
                st["s2"] = s2
            else:
                for s in range(nsub):
                    tp = fpsum.tile([sub, FC2_OUT], f32, tag="fps",
                                    name=f"tp_{bt}_{s}")
                    nc.tensor.transpose(
                        tp[:], st["s2"][:, s * sub : (s + 1) * sub], ident[:]
                    )
                    e = smpool.tile([sub, FC2_OUT], f32, tag="e")
                    nc.scalar.activation(
                        e[:], tp[:], mybir.ActivationFunctionType.Exp,
                    )
                    ssum = smpool.tile([sub, 1], f32, tag="ss")
                    nc.vector.tensor_reduce(
                        ssum[:], e[:], axis=mybir.AxisListType.X,
                        op=mybir.AluOpType.add,
                    )
                    rinv = smpool.tile([sub, 1], f32, tag="ri")
                    nc.vector.reciprocal(rinv[:], ssum[:])
                    c0 = (bt * nsub + s) * FC2_OUT
                    # scale-by-AP copy on scalar: keeps the softmax chain
                    # split across scalar (exp, mul) and vector (sum, recip)
                    nc.scalar.mul(
                        stage[:, c0 : c0 + FC2_OUT], e[:], rinv[:]
                    )
                if bt == nbt - 1:
                    nc.sync.dma_start(o_d[:], stage[:])

        tails = {}
        for bt in range(nbt):
            b0 = bt * nb
            a1 = [None] * KB
            st = {}
            tails[bt] = st
            for kb in range(KB + SKEW):
                # ration the remaining fc1 weight groups: group g is first
                # read at kb=4g+SKEW, so issuing at 4g-2 keeps the stream
                # one group ahead without starving the gather queue early
                if bt == 0 and kb >= 2 and (kb + 2) % WG == 0:
                    g_id = (kb + 2) // WG
                    if g_id < KB // WG:
                        nc.gpsimd.dma_start(w1g[g_id][:], w1_d[g_id])
                # inject the previous tile's fc2/softmax into this tile's
                # conv-only window (kb < SKEW, PE has 4-matmul slack)
                if bt > 0 and 1 <= kb <= 3:
                    emit_tail(bt - 1, kb - 1, tails[bt - 1])
                if kb >= SKEW:
                    j = kb - SKEW
                    if j == 0:
                        # fc1 accumulators; allocated here so the previous
                        # tile's tail (p2f/tp) can rotate out first
                        st["fp"] = [
                            fpsum.tile([MTS, nb], f32, tag="fps",
                                       name=f"fp{bt}_{mt}")
                            for mt in range(MT)
                        ]
                    for mt in range(MT):
                        nc.tensor.matmul(
                            st["fp"][mt][:],
                            w1_slice(j, mt),
                            a1[j][:],
                            start=(j == 0),
                            stop=(j == KB - 1),
                        )
                if kb >= KB:
                    continue
                ps = [
                    cpsum.tile([KBS, nb], f32, tag="cps", name=f"cps{i}")
                    for i in range(4)
                ]
                for dr in range(2):
                    for eo in range(2):
                        nc.tensor.matmul(
                            ps[2 * dr + eo][:],
                            t4[:, (2 * dr + eo) * KBS : (2 * dr + eo + 1) * KBS],
                            gx[bt][kb][:],
                            start=True,
                            stop=True,
                        )
                # 2x2 maxpool: scalar evacuates the even-parity PSUM banks,
                # vector does the width maxes (PSUM-limited 1x) writing
                # bf16, so the final height max runs in the DVE's 2x_1P
                # packed mode at half cost.
                s0 = tmppool.tile([KBS, nb], f32, tag="s")
                nc.scalar.copy(s0[:], ps[0][:])
                m0 = tmppool.tile([KBS, nb], bf16, tag="m")
                nc.vector.tensor_max(m0[:], s0[:], ps[1][:])
                s1 = tmppool.tile([KBS, nb], f32, tag="s")
                nc.scalar.copy(s1[:], ps[2][:])
                m1 = tmppool.tile([KBS, nb], bf16, tag="m")
                nc.vector.tensor_max(m1[:], s1[:], ps[3][:])
                ab = a1pool.tile([KBS, nb], bf16, tag="a1")
                nc.vector.tensor_max(ab[:], m0[:], m1[:])
                a1[kb] = ab

        for stage_no in range(3):
            emit_tail(nbt - 1, stage_no, tails[nbt - 1])

    nc.compile()
    return nc


def _prep_weights(conv_w, conv_b, fc1_w, fc1_b, fc2_w, fc2_b):
    conv_w = np.asarray(conv_w, np.float32).reshape(COUT, KS, KS)
    conv_b = np.asarray(conv_b, np.float32)
    fc1_w = np.asarray(fc1_w, np.float32)
    fc1_b = np.asarray(fc1_b, np.float32)
    fc2_w = np.asarray(fc2_w, np.float32)
    fc2_b = np.asarray(fc2_b, np.float32)

    # Toeplitz conv matrices [96, 4*120]: four stationaries (dr, eo) over a
    # merged 6-row x 16-col gather; col m = (2*dr+eo)*120 + c*6 + q maps to
    # conv output (row 2*ip+dr, col 12*jb + 2*q+eo, channel c).
    T4 = np.zeros((GROWS, 4 * KBS), np.float32)
    for dr in range(2):
        for eo in range(2):
            for c in range(COUT):
                for q in range(6):
                    m = (2 * dr + eo) * KBS + c * 6 + q
                    for di in range(KS):
                        for dj in range(KS):
                            T4[(di + dr) * 16 + 2 * q + eo + dj, m] = conv_w[c, di, dj]

    # fc1 weights permuted to our pooled-feature order:
    # block kb = ip*2 + jb, within-block m = c*6 + q
    # -> original flat feature c*144 + ip*12 + jb*6 + q
    kbv = np.arange(KB)
    ipv, jbv = kbv // 2, kbv % 2
    ml = np.arange(KBS)
    cv, qv = ml // 6, ml % 6
    fidx = cv[None, :] * 144 + ipv[:, None] * 12 + jbv[:, None] * 6 + qv[None, :]
    w1 = fc1_w.T[fidx.reshape(-1)].reshape(KB, KBS, FC1_OUT)
    # pack into 6 contiguous groups of 4 blocks: [6, 120, 4*500]
    w1 = np.ascontiguousarray(
        w1.reshape(KB // 4, 4, KBS, FC1_OUT).transpose(0, 2, 1, 3)
    ).reshape(KB // 4, KBS, 4 * FC1_OUT)

    # conv bias folded into fc1 bias (pool-max commutes with per-channel const)
    cb_vec = np.repeat(conv_b, 144)
    b1p = fc1_b + fc1_w @ cb_vec
    b1 = np.ascontiguousarray(b1p.reshape(MT, MTS).T)

    w2 = np.ascontiguousarray(
        fc2_w.T.reshape(MT, MTS, FC2_OUT).transpose(1, 0, 2)
    ).reshape(MTS, MT * FC2_OUT)
    b2 = np.ascontiguousarray(fc2_b.reshape(FC2_OUT, 1))
    return (T4.astype(BF16), w1.astype(BF16), b1,
            w2.astype(BF16), b2)


# im2col pixel indices: idx[jb, p, ip] = (2*ip + p//16)*28 + 12*jb + p%16
_IDX = np.zeros((2, GROWS, PH), np.int64)
for _jb in range(2):
    for _di in range(6):
        for _jjp in range(16):
            for _ip in range(PH):
                _IDX[_jb, _di * 16 + _jjp, _ip] = (2 * _ip + _di) * W + 12 * _jb + _jjp


def _prep_x(x_core):
    """x_core [784, npc] pixel-major -> xg [2, 96, 12, npc] bf16."""
    return np.ascontiguousarray(x_core[_IDX.reshape(-1)].reshape(
        2, GROWS, PH, x_core.shape[1]).astype(BF16))


def _feeds(inputs, npc):
    """Per-core feed dicts for the full batch (list of NCORES dicts)."""
    T4, w1, b1, w2, b2 = _prep_weights(
        inputs["conv_w"], inputs["conv_b"], inputs["fc1_w"],
        inputs["fc1_b"], inputs["fc2_w"], inputs["fc2_b"],
    )
    x = np.asarray(inputs["x"], np.float32).reshape(-1, H * W)
    n_total = x.shape[0]
    assert n_total == NCORES * npc
    xs = x.reshape(NCORES, npc, H * W).transpose(0, 2, 1)
    return [
        {"xg": _prep_x(xs[i]), "t4": T4, "w1": w1, "b1": b1, "w2": w2,
         "b2": b2}
        for i in range(NCORES)
    ]


def _untangle_out(o, npc, nb):
    """Device staging layout [sub, nbt*nsub*10] -> [npc, 10]."""
    sub = min(128, nb)
    nsub = nb // sub
    nbt = npc // nb
    return np.ascontiguousarray(
        np.asarray(o).reshape(sub, nbt, nsub, FC2_OUT)
        .transpose(1, 2, 0, 3).reshape(npc, FC2_OUT)
    )


def _run(inputs, npc=NPC, nb=512, trace=False):
    from concourse import bass_utils

    key = (npc, nb)
    if key not in _cache:
        _cache[key] = _build(npc, nb)
    nc = _cache[key]

    in_maps = _feeds(inputs, npc)
    res = bass_utils.run_bass_kernel_spmd(
        nc, in_maps, core_ids=list(range(NCORES)), trace=trace
    )
    out = np.concatenate(
        [_untangle_out(res.results[i]["out"], npc, nb) for i in range(NCORES)],
        axis=0,
    )
    return out, res


def kernel(**inputs):
    out, _ = _run(inputs)
    return out


# revision 30
# speedup vs baseline: 1.0665x; 1.0200x over previous
"""Trainium2 Bass kernel for nn_CNNVectorForm (LeNet-style CNN, batch 8192).

Pipeline per core (data-parallel over batch, 1024 images/core):
  conv 5x5 VALID (1->20ch, 28->24)  -> 2x2 maxpool -> fc1(2880->500) + relu
  -> fc2(500->10) + softmax

Device formulation (v2, bf16 datapath):
  * All activations feature-major [features, batch]; batch rides the free
    dim (512 per tile).  Weights and activations are bf16 (1 cycle/row on
    the PE, half the HBM traffic of fp32); PSUM accumulation is fp32.
  * Conv as a Toeplitz matmul over merged 6-row gathers: per pooled row
    ip and column half jb one [96, nb] gather (6 input rows x 16 cols)
    feeds 4 matmuls (output row parity dr x column parity eo) with
    zero-padded stationaries T4[dr,eo] [96, 120].  Merging the rows cuts
    gather traffic 40% and halves the DMA instruction count vs per-row
    gathers.
  * 2x2 maxpool: scalar engine evacuates ps0/ps2 (PSUM->SBUF), vector
    does the two width maxes, gpsimd (idle otherwise) does the final
    height max, writing a1 in bf16.  Three engines run in parallel and
    each stays under the PE's 8-matmul-per-kb budget.
  * fc1 weights host-permuted to pooled-feature order; fc1 rides the conv
    loop skewed by SKEW blocks (4 dependency-free matmuls per quad).
  * conv bias folded into the fc1 bias on the host.
  * fc2 feature-major (4 accumulating K=125 matmuls), bias via vector
    tensor_scalar_add, PE-transpose 128-wide slices for the softmax,
    results staged in one [128, 40] tile -> single output DMA per tile.
"""

import numpy as np
import ml_dtypes

N, H, W = 8192, 28, 28
COUT, KS = 20, 5
NCORES = 8
NPC = N // NCORES  # images per core
PH = 12            # pooled rows
FC1_IN, FC1_OUT, FC2_OUT = 2880, 500, 10
MT, MTS = 4, 125   # fc1 M tiles
KB, KBS = 24, 120  # a1 feature blocks (one per (pooled row, column half))
SKEW = 6           # fc1 trails conv by SKEW blocks
GROWS = 96         # merged gather rows: 6 input rows x 16 cols

BF16 = ml_dtypes.bfloat16

_cache = {}


def _build(npc, nb):
    from contextlib import ExitStack

    import concourse.tile as tile
    from concourse import bacc, mybir

    f32 = mybir.dt.float32
    bf16 = mybir.dt.bfloat16
    nbt = npc // nb

    nc = bacc.Bacc(
        "TRN2",
        target_bir_lowering=False,
        debug=False,
        enable_asserts=False,
        num_devices=NCORES,
    )

    # host-im2col'd input: xg[jb, p, ip, b] = x[(2*ip + p//16)*28 + 12*jb + p%16, b]
    CHK = 4  # conv blocks per gather chunk; 4KB descriptor lines
    xg_d = nc.dram_tensor(
        "xg", [nbt, GROWS, KB * nb], bf16, kind="ExternalInput"
    ).ap()
    t4_d = nc.dram_tensor("t4", [GROWS, 4 * KBS], bf16, kind="ExternalInput").ap()
    w1_d = nc.dram_tensor(
        "w1", [KB // 4, KBS, 4 * FC1_OUT], bf16, kind="ExternalInput"
    ).ap()
    b1_d = nc.dram_tensor("b1", [MTS, MT], f32, kind="ExternalInput").ap()
    w2_d = nc.dram_tensor("w2", [MTS, MT * FC2_OUT], bf16, kind="ExternalInput").ap()
    b2_d = nc.dram_tensor("b2", [FC2_OUT, 1], f32, kind="ExternalInput").ap()
    # output stays in the device-native [128, nbt*nsub*10] staging layout;
    # the host untangles it (fewer, bigger DMA descriptors)
    sub = min(128, nb)
    nsub = nb // sub
    o_d = nc.dram_tensor(
        "out", [sub, nbt * nsub * FC2_OUT], f32, kind="ExternalOutput"
    ).ap()

    with tile.TileContext(nc) as tc, ExitStack() as ctx:
        const = ctx.enter_context(tc.tile_pool(name="const", bufs=1))
        w1pool = ctx.enter_context(tc.tile_pool(name="w1", bufs=6))
        gpool = ctx.enter_context(tc.tile_pool(name="gather", bufs=1))
        a1pool = ctx.enter_context(tc.tile_pool(name="a1", bufs=SKEW + 4))
        tmppool = ctx.enter_context(tc.tile_pool(name="ptmp", bufs=6))
        a2pool = ctx.enter_context(tc.tile_pool(name="a2", bufs=2 * MT))
        smpool = ctx.enter_context(tc.tile_pool(name="softmax", bufs=4))
        cpsum = ctx.enter_context(tc.tile_pool(name="cpsum", bufs=4, space="PSUM"))
        fpsum = ctx.enter_context(tc.tile_pool(name="fpsum", bufs=4, space="PSUM"))

        from concourse.masks import make_identity

        # The DMA arbiter round-robins DESCRIPTORS across queues, so byte
        # share is proportional to descriptor line size.  The gathers are
        # host-packed into per-partition-contiguous chunks of CHK conv
        # blocks (4KB lines, matching the w1 stream) and issued in priority
        # order: the bytes gating the first conv matmul first, then tile
        # 0's chunks, then tile 1's.  w1 group 0 goes upfront on gpsimd's
        # queue; the rest are rationed inside the loop.
        t4 = const.tile([GROWS, 4 * KBS], bf16)
        nc.sync.dma_start(t4[:, 0 : 2 * KBS], t4_d[:, 0 : 2 * KBS])
        nc.sync.dma_start(t4[:, 2 * KBS :], t4_d[:, 2 * KBS :])
        gxc = [[None] * (KB // CHK) for _ in range(nbt)]
        for bt in range(nbt):
            for c in range(KB // CHK):
                g = gpool.tile([GROWS, CHK * nb], bf16, tag=f"gc{bt}_{c}")
                nc.sync.dma_start(
                    g[:], xg_d[bt, :, c * CHK * nb : (c + 1) * CHK * nb]
                )
                gxc[bt][c] = g
        WG = 4
        w1g = []
        for gidx in range(KB // WG):
            wt = w1pool.tile([KBS, WG * FC1_OUT], bf16, tag="w1",
                             name=f"w1g{gidx}")
            if gidx == 0:
                nc.gpsimd.dma_start(wt[:], w1_d[gidx])
            w1g.append(wt)
        b1t = const.tile([MTS, MT], f32)
        nc.scalar.dma_start(b1t[:], b1_d[:])
        w2t = const.tile([MTS, MT * FC2_OUT], bf16)
        nc.scalar.dma_start(w2t[:], w2_d[:])
        b2t = const.tile([FC2_OUT, 1], f32)
        nc.scalar.dma_start(b2t[:], b2_d[:])
        ident = const.tile([FC2_OUT, FC2_OUT], f32)
        make_identity(nc, ident[:])

        def w1_slice(j, mt):
            return w1g[j // WG][
                :, (j % WG) * FC1_OUT + mt * MTS : (j % WG) * FC1_OUT + (mt + 1) * MTS
            ]

        # PE warmup: HAM un-throttles the PE clock (1.2 -> 2.4 GHz) only
        # after ~3.4us of sustained matmul activity.  Real data needs
        # ~12us of DMA before the first conv matmul; a memset operand
        # gives the PE junk matmuls to chew on meanwhile so every real
        # matmul runs at full clock.
        warm = const.tile([128, nb], bf16)
        nc.vector.memset(warm[:], 0.0)
        for wi in range(16):
            wps = cpsum.tile([min(128, nb), nb], f32, tag="cps",
                             name=f"warm{wi}")
            nc.tensor.matmul(wps[:], warm[:, 0 : min(128, nb)], warm[:],
                             start=True, stop=True)

        stage = const.tile([sub, nbt * nsub * FC2_OUT], f32)

        def emit_tail(bt, stage_no, st):
            """Tail of tile bt, split in 3 stages so it interleaves with
            the next tile's conv-only (pool-paced) window."""
            b0 = bt * nb
            if stage_no == 0:
                st["a2"] = []
                for mt in range(MT):
                    a2 = a2pool.tile([MTS, nb], bf16, tag="a2")
                    if bt == nbt - 1 and mt % 2 == 1:
                        # final tile: the relu chain is exposed tail
                        # latency; split it across scalar and vector
                        # (fused (fp + b1) max 0 in one DVE op)
                        nc.vector.tensor_scalar(
                            a2[:], st["fp"][mt][:],
                            b1t[:, mt : mt + 1], 0.0,
                            mybir.AluOpType.add, mybir.AluOpType.max,
                        )
                    else:
                        nc.scalar.activation(
                            a2[:], st["fp"][mt][:],
                            mybir.ActivationFunctionType.Relu,
                            bias=b1t[:, mt : mt + 1],
                        )
                    st["a2"].append(a2)
            elif stage_no == 1:
                p2f = fpsum.tile([FC2_OUT, nb], f32, tag="fps",
                                 name=f"p2f_{bt}")
                for mt in range(MT):
                    nc.tensor.matmul(
                        p2f[:],
                        w2t[:, mt * FC2_OUT : (mt + 1) * FC2_OUT],
                        st["a2"][mt][:],
                        start=(mt == 0),
                        stop=(mt == MT - 1),
                    )
                s2 = smpool.tile([FC2_OUT, nb], f32, tag="s2")
                if bt == nbt - 1:
                    # final tile: the DVE is busy with the relu split; the
                    # scalar PSUM->SBUF path is faster and idle here
                    nc.scalar.activation(
                        s2[:], p2f[:],
                        mybir.ActivationFunctionType.Identity,
                        bias=b2t[:, 0:1],
                    )
                else:
                    nc.vector.tensor_scalar_add(s2[:], p2f[:], b2t[:, 0:1])
                st["s2"] = s2
            else:
                for s in range(nsub):
                    tp = fpsum.tile([sub, FC2_OUT], f32, tag="fps",
                                    name=f"tp_{bt}_{s}")
                    nc.tensor.transpose(
                        tp[:], st["s2"][:, s * sub : (s + 1) * sub], ident[:]
                    )
                    e = smpool.tile([sub, FC2_OUT], f32, tag="e")
                    nc.scalar.activation(
                        e[:], tp[:], mybir.ActivationFunctionType.Exp,
                    )
                    ssum = smpool.tile([sub, 1], f32, tag="ss")
                    nc.vector.tensor_reduce(
                        ssum[:], e[:], axis=mybir.AxisListType.X,
                        op=mybir.AluOpType.add,
                    )
                    rinv = smpool.tile([sub, 1], f32, tag="ri")
                    nc.vector.reciprocal(rinv[:], ssum[:])
                    c0 = (bt * nsub + s) * FC2_OUT
                    # alternate the normalize between vector and scalar so
                    # neither engine serializes the whole chunk chain
                    if s % 2 == 0:
                        nc.vector.tensor_scalar_mul(
                            stage[:, c0 : c0 + FC2_OUT], e[:], rinv[:]
                        )
                    else:
                        nc.scalar.mul(
                            stage[:, c0 : c0 + FC2_OUT], e[:], rinv[:]
                        )
                w0 = bt * nsub * FC2_OUT
                nc.sync.dma_start(
                    o_d[:, w0 : w0 + nsub * FC2_OUT],
                    stage[:, w0 : w0 + nsub * FC2_OUT],
                )

        tails = {}
        for bt in range(nbt):
            b0 = bt * nb
            a1 = [None] * KB
            st = {}
            tails[bt] = st
            for kb in range(KB + SKEW):
                # ration the remaining fc1 weight groups: group g is first
                # read at kb=4g+SKEW, so issuing at 4g-2 keeps the stream
                # one group ahead without starving the gather queue early
                if bt == 0 and kb >= 2 and (kb + 2) % WG == 0:
                    g_id = (kb + 2) // WG
                    if g_id < KB // WG:
                        nc.gpsimd.dma_start(w1g[g_id][:], w1_d[g_id])
                # inject the previous tile's fc2/softmax into this tile's
                # conv-only window (kb < SKEW, PE has 4-matmul slack)
                if bt > 0 and 1 <= kb <= 3:
                    emit_tail(bt - 1, kb - 1, tails[bt - 1])
                if kb >= SKEW:
                    j = kb - SKEW
                    if j == 0:
                        # fc1 accumulators; allocated here so the previous
                        # tile's tail (p2f/tp) can rotate out first
                        st["fp"] = [
                            fpsum.tile([MTS, nb], f32, tag="fps",
                                       name=f"fp{bt}_{mt}")
                            for mt in range(MT)
                        ]
                    for mt in range(MT):
                        nc.tensor.matmul(
                            st["fp"][mt][:],
                            w1_slice(j, mt),
                            a1[j][:],
                            start=(j == 0),
                            stop=(j == KB - 1),
                        )
                if kb >= KB:
                    continue
                ps = [
                    cpsum.tile([KBS, nb], f32, tag="cps", name=f"cps{i}")
                    for i in range(4)
                ]
                gs = gxc[bt][kb // CHK][:, (kb % CHK) * nb : (kb % CHK + 1) * nb]
                for dr in range(2):
                    for eo in range(2):
                        nc.tensor.matmul(
                            ps[2 * dr + eo][:],
                            t4[:, (2 * dr + eo) * KBS : (2 * dr + eo + 1) * KBS],
                            gs,
                            start=True,
                            stop=True,
                        )
                # 2x2 maxpool: scalar evacuates the even-parity PSUM banks,
                # vector does the width maxes (PSUM-limited 1x) writing
                # bf16, so the final height max runs in the DVE's 2x_1P
                # packed mode at half cost.
                s0 = tmppool.tile([KBS, nb], f32, tag="s")
                nc.scalar.copy(s0[:], ps[0][:])
                m0 = tmppool.tile([KBS, nb], bf16, tag="m")
                nc.vector.tensor_max(m0[:], s0[:], ps[1][:])
                s1 = tmppool.tile([KBS, nb], f32, tag="s")
                nc.scalar.copy(s1[:], ps[2][:])
                m1 = tmppool.tile([KBS, nb], bf16, tag="m")
                nc.vector.tensor_max(m1[:], s1[:], ps[3][:])
                ab = a1pool.tile([KBS, nb], bf16, tag="a1")
                nc.vector.tensor_max(ab[:], m0[:], m1[:])
                a1[kb] = ab

        for stage_no in range(3):
            emit_tail(nbt - 1, stage_no, tails[nbt - 1])

    nc.compile()
    return nc


def _prep_weights(conv_w, conv_b, fc1_w, fc1_b, fc2_w, fc2_b):
    conv_w = np.asarray(conv_w, np.float32).reshape(COUT, KS, KS)
    conv_b = np.asarray(conv_b, np.float32)
    fc1_w = np.asarray(fc1_w, np.float32)
    fc1_b = np.asarray(fc1_b, np.float32)
    fc2_w = np.asarray(fc2_w, np.float32)
    fc2_b = np.asarray(fc2_b, np.float32)

    # Toeplitz conv matrices [96, 4*120]: four stationaries (dr, eo) over a
    # merged 6-row x 16-col gather; col m = (2*dr+eo)*120 + c*6 + q maps to
    # conv output (row 2*ip+dr, col 12*jb + 2*q+eo, channel c).
    T4 = np.zeros((GROWS, 4 * KBS), np.float32)
    for dr in range(2):
        for eo in range(2):
            for c in range(COUT):
                for q in range(6):
                    m = (2 * dr + eo) * KBS + c * 6 + q
                    for di in range(KS):
                        for dj in range(KS):
                            T4[(di + dr) * 16 + 2 * q + eo + dj, m] = conv_w[c, di, dj]

    # fc1 weights permuted to our pooled-feature order:
    # block kb = ip*2 + jb, within-block m = c*6 + q
    # -> original flat feature c*144 + ip*12 + jb*6 + q
    kbv = np.arange(KB)
    ipv, jbv = kbv // 2, kbv % 2
    ml = np.arange(KBS)
    cv, qv = ml // 6, ml % 6
    fidx = cv[None, :] * 144 + ipv[:, None] * 12 + jbv[:, None] * 6 + qv[None, :]
    w1 = fc1_w.T[fidx.reshape(-1)].reshape(KB, KBS, FC1_OUT)
    # pack into 6 contiguous groups of 4 blocks: [6, 120, 4*500]
    w1 = np.ascontiguousarray(
        w1.reshape(KB // 4, 4, KBS, FC1_OUT).transpose(0, 2, 1, 3)
    ).reshape(KB // 4, KBS, 4 * FC1_OUT)

    # conv bias folded into fc1 bias (pool-max commutes with per-channel const)
    cb_vec = np.repeat(conv_b, 144)
    b1p = fc1_b + fc1_w @ cb_vec
    b1 = np.ascontiguousarray(b1p.reshape(MT, MTS).T)

    w2 = np.ascontiguousarray(
        fc2_w.T.reshape(MT, MTS, FC2_OUT).transpose(1, 0, 2)
    ).reshape(MTS, MT * FC2_OUT)
    b2 = np.ascontiguousarray(fc2_b.reshape(FC2_OUT, 1))
    return (T4.astype(BF16), w1.astype(BF16), b1,
            w2.astype(BF16), b2)


# im2col pixel indices: idx[kb, p] = (2*(kb//2) + p//16)*28 + 12*(kb%2) + p%16
_IDX = np.zeros((KB, GROWS), np.int64)
for _kb in range(KB):
    for _di in range(6):
        for _jjp in range(16):
            _IDX[_kb, _di * 16 + _jjp] = (
                (2 * (_kb // 2) + _di) * W + 12 * (_kb % 2) + _jjp
            )


def _prep_x(x_core, nb):
    """x_core [784, npc] pixel-major -> xg [nbt, 96, KB*nb] bf16,
    per-partition contiguous per batch tile (big DMA descriptor lines)."""
    npc = x_core.shape[1]
    nbt = npc // nb
    g = x_core[_IDX.reshape(-1)].reshape(KB, GROWS, nbt, nb)
    return np.ascontiguousarray(
        g.transpose(2, 1, 0, 3).reshape(nbt, GROWS, KB * nb).astype(BF16)
    )


def _feeds(inputs, npc, nb):
    """Per-core feed dicts for the full batch (list of NCORES dicts)."""
    T4, w1, b1, w2, b2 = _prep_weights(
        inputs["conv_w"], inputs["conv_b"], inputs["fc1_w"],
        inputs["fc1_b"], inputs["fc2_w"], inputs["fc2_b"],
    )
    x = np.asarray(inputs["x"], np.float32).reshape(-1, H * W)
    n_total = x.shape[0]
    assert n_total == NCORES * npc
    xs = x.reshape(NCORES, npc, H * W).transpose(0, 2, 1)
    return [
        {"xg": _prep_x(xs[i], nb), "t4": T4, "w1": w1, "b1": b1, "w2": w2,
         "b2": b2}
        for i in range(NCORES)
    ]


def _untangle_out(o, npc, nb):
    """Device staging layout [sub, nbt*nsub*10] -> [npc, 10]."""
    sub = min(128, nb)
    nsub = nb // sub
    nbt = npc // nb
    return np.ascontiguousarray(
        np.asarray(o).reshape(sub, nbt, nsub, FC2_OUT)
        .transpose(1, 2, 0, 3).reshape(npc, FC2_OUT)
    )


def _run(inputs, npc=NPC, nb=512, trace=False):
    from concourse import bass_utils

    key = (npc, nb)
    if key not in _cache:
        _cache[key] = _build(npc, nb)
    nc = _cache[key]

    in_maps = _feeds(inputs, npc, nb)
    res = bass_utils.run_bass_kernel_spmd(
        nc, in_maps, core_ids=list(range(NCORES)), trace=trace
    )
    out = np.concatenate(
        [_untangle_out(res.results[i]["out"], npc, nb) for i in range(NCORES)],
        axis=0,
    )
    return out, res


def kernel(**inputs):
    out, _ = _run(inputs)
    return out
